# revision 39
# baseline (speedup 1.0000x reference)
"""Trainium2 Bass kernel for cross-attention (b=2, n=2048, dim=512, 8 heads x 64).

Sharding: batch*heads across 8 cores. Core c handles batch c//4 and heads
{2*(c%4), 2*(c%4)+1} (a contiguous 128-col slice of the INNER dim).
Host pre-transposes activations to channel-major so the PE contraction dim
lands on partitions with fully-contiguous DMA.

Per-core dataflow (fp16 DMA/projection inputs, float32r PE streaming at
1 cycle/row, fp32 PSUM accumulation):
  qT/kT/vT = (W^T x) as [128 dcols, 2048 rows]; v re-transposed to [j, d]
  blocks via PE transpose, augmented with a ones column (denominator trick).
  Attention works on transposed scores so softmax needs no cross-partition
  reduction: scoresT[j,i] -> exp on ScalarE -> AV accumulate with
  lhsT=[v_h | 1] giving [64 d + 1 denom row, i] in PSUM.
  Normalize = reciprocal + K=1 ones-matmul broadcast + one multiply.
  Out-proj: partial = oT^T @ Wo_slice; host sums 4 partials/batch + bias.

Emission is a position-scheduled software pipeline paced by ScalarE (the
bottleneck engine): each position carries one scores-pair + one exp op;
AV matmuls trail by AV_OFF positions so the FIFO PE stream never blocks on
ScalarE or on late DMA arrivals; projections, v transposes, normalize and
out-proj blocks are slotted where their DMA/data dependencies land.
"""

import os
import numpy as np

DIM = 512
HEADS = 8
DH = 64
B = 2
N = 2048
NCORES = 8
SLICE = 128  # INNER columns per core (2 heads)

_CACHE = {}


def _build_nc():
    import concourse.bass as bass
    import concourse.mybir as mybir
    import concourse.tile as tile
    from concourse import bacc
    from concourse.masks import make_identity

    f32 = mybir.dt.float32
    f32r = mybir.dt.float32r
    f16 = mybir.dt.float16
    EXP = mybir.ActivationFunctionType.Exp

    nc = bacc.Bacc("TRN2", target_bir_lowering=False, debug=False)

    xT = nc.dram_tensor("xT", [DIM, N], f16, kind="ExternalInput").ap()
    mT = nc.dram_tensor("mT", [DIM, N], f16, kind="ExternalInput").ap()
    yT = nc.dram_tensor("yT", [DIM, N], f16, kind="ExternalInput").ap()
    wq = nc.dram_tensor("wq", [DIM, SLICE], f16, kind="ExternalInput").ap()
    wk = nc.dram_tensor("wk", [DIM, SLICE], f16, kind="ExternalInput").ap()
    wv = nc.dram_tensor("wv", [DIM, SLICE], f16, kind="ExternalInput").ap()
    wo = nc.dram_tensor("wo", [SLICE, DIM], f32r, kind="ExternalInput").ap()
    out = nc.dram_tensor("out", [N, DIM], f16, kind="ExternalOutput").ap()

    with tile.TileContext(nc) as tc:
        with (
            tc.tile_pool(name="wpool", bufs=1) as wpool,
            tc.tile_pool(name="cin1", bufs=16) as cin1,
            tc.tile_pool(name="persist", bufs=1) as persist,
            tc.tile_pool(name="expool", bufs=15) as expool,
            tc.tile_pool(name="snpool", bufs=2) as snpool,
            tc.tile_pool(name="obpool", bufs=3) as obpool,
            tc.tile_pool(name="mmp", bufs=2, space="PSUM") as mmp,
            tc.tile_pool(name="avp", bufs=4, space="PSUM") as avp,
        ):
            # ---- persistent SBUF tensors ----
            wq_sb = wpool.tile([128, DIM], f16, tag="wq")
            wk_sb = wpool.tile([128, DIM], f16, tag="wk")
            wv_sb = wpool.tile([128, DIM], f16, tag="wv")
            wo0_sb = wpool.tile([64, DIM], f32r, tag="wo0")
            wo1_sb = wpool.tile([64, DIM], f32r, tag="wo1")
            qT = persist.tile([128, N], f32r, tag="qT")
            kT = persist.tile([128, N], f32r, tag="kT")
            vT = persist.tile([128, N], f32r, tag="vT")
            ident_f = persist.tile([128, 128], f32, tag="ident_f")
            ident = persist.tile([128, 128], f32r, tag="ident")
            v_all = persist.tile([128, 16 * 130], f32r, tag="v_all")
            oT0 = persist.tile([64, N], f32r, tag="oT0")
            oT1 = persist.tile([64, N], f32r, tag="oT1")
            ones65 = persist.tile([65, 64], f32r, tag="ones65")
            ones_f = persist.tile([128, 64], f32, tag="ones_f")
            zbias = persist.tile([128, 1], f32, tag="zbias")

            nc.vector.memset(ones_f[:, :], 1.0)
            nc.vector.memset(zbias[:, :], 0.0)
            nc.vector.tensor_copy(ones65[:, :], ones_f[0:65, :])
            nc.vector.tensor_copy(v_all[:, 64::65], ones_f[:, 0:32])
            make_identity(nc, ident_f[:, :])
            nc.vector.tensor_copy(ident[:, :], ident_f[:, :])

            def load_w(w_dram, w_sb):
                nc.sync.dma_start(
                    w_sb[:, :].rearrange("p (c m) -> p c m", c=4),
                    w_dram.rearrange("(c p) m -> p c m", p=128),
                )

            # ---- input DMAs: column-halves, queue order = priority ----
            # order: m[col0], x[col0], y[col0], y[col1], m[col1], x[col1]
            def load_half(src_ap, col, name):
                tiles = []
                for ck in range(4):
                    t = cin1.tile([128, 1024], f16, tag="cin", name=f"{name}{col}_{ck}")
                    nc.sync.dma_start(t[:, :], src_ap[ck * 128:(ck + 1) * 128,
                                                      col * 1024:(col + 1) * 1024])
                    tiles.append(t)
                return tiles

            def load_quarter(src_ap, q, name):
                tiles = []
                for ck in range(4):
                    t = cin1.tile([128, 512], f16, tag="cinq", name=f"{name}q{q}_{ck}")
                    nc.sync.dma_start(t[:, :], src_ap[ck * 128:(ck + 1) * 128,
                                                      q * 512:(q + 1) * 512])
                    tiles.append(t)
                return tiles

            load_w(wk, wk_sb)
            m00 = load_quarter(mT, 0, "m")
            load_w(wq, wq_sb)
            x00 = load_quarter(xT, 0, "x")
            m01 = load_quarter(mT, 1, "m")
            x01 = load_quarter(xT, 1, "x")
            load_w(wv, wv_sb)
            y0 = load_half(yT, 0, "y")
            m1 = load_half(mT, 1, "m")
            y1 = load_half(yT, 1, "y")
            x1 = load_half(xT, 1, "x")
            nc.sync.dma_start(wo0_sb[:, :], wo[0:64, :])
            nc.sync.dma_start(wo1_sb[:, :], wo[64:128, :])

            # ---- building blocks ----
            def proj_qk(dst, w_sb, chunks, nb, chunk_col_off):
                ps = avp.tile([128, 512], f32, tag="av", name=f"ps_p{dst.tensor.name}_{nb}")
                for ck in range(4):
                    nc.tensor.matmul(
                        ps[:, :],
                        lhsT=w_sb[:, ck * 128:(ck + 1) * 128],
                        rhs=chunks[ck][:, chunk_col_off:chunk_col_off + 512],
                        start=(ck == 0), stop=(ck == 3),
                    )
                nc.vector.tensor_copy(dst[:, nb * 512:(nb + 1) * 512], ps[:, :])

            def v_transpose(jb):
                pv = avp.tile([128, 512], f32r, tag="av", name=f"pv{jb}")
                nc.tensor.transpose(pv[:, 0:128], vT[:, jb * 128:(jb + 1) * 128], ident[:, :])
                base = jb * 130
                nc.vector.tensor_copy(v_all[:, base:base + 64], pv[:, 0:64])
                nc.vector.tensor_copy(v_all[:, base + 65:base + 129], pv[:, 64:128])

            # per-unit state: unit = (ip, ihalf); the 1024-wide psum/exp tile
            # packs [head0 | head1] for one 512-wide i-block, so the two score
            # matmuls sit on disjoint PE row groups (base partitions 0/64) and
            # overlap in the array.
            units = [(0, 0), (0, 1), (1, 0), (1, 1)]
            av_tiles = {}   # ui -> [av0, av1]
            ex_tiles = {}   # (ui, jb) -> exp tile
            norm_state = {}

            def emit_scores_exp(ui, jb):
                ip, ihalf = units[ui]
                icol = ip * 1024 + ihalf * 512
                ps = mmp.tile([128, 1024], f32, tag="mm", name=f"ps_s{ui}_{jb}")
                for h in range(2):
                    hp = h * 64
                    nc.tensor.matmul(
                        ps[:, h * 512:(h + 1) * 512],
                        lhsT=kT[hp:hp + 64, jb * 128:(jb + 1) * 128],
                        rhs=qT[hp:hp + 64, icol:icol + 512],
                        start=True, stop=True,
                    )
                ex = expool.tile([128, 1024], f32r, tag="ex", name=f"ex{ui}_{jb}")
                nc.scalar.activation(ex[:, :], ps[:, :], EXP, bias=zbias[:, 0:1])
                ex_tiles[(ui, jb)] = ex

            def emit_av(ui, jb):
                if jb == 0:
                    av_tiles[ui] = [
                        avp.tile([65, 512], f32, tag="av", name=f"av{ui}_{h}")
                        for h in range(2)
                    ]
                av = av_tiles[ui]
                ex = ex_tiles.pop((ui, jb))
                for h in range(2):
                    vb = jb * 130 + h * 65
                    nc.tensor.matmul(
                        av[h][:, :],
                        lhsT=v_all[:, vb:vb + 65],
                        rhs=ex[:, h * 512:(h + 1) * 512],
                        start=(jb == 0), stop=(jb == 15),
                        skip_group_check=True,
                    )

            def normalize_a(ui):
                av = av_tiles[ui]
                un = snpool.tile([65, 1024], f32, tag="un", name=f"un{ui}")
                rc = snpool.tile([65, 1024], f32r, tag="rc", name=f"rc{ui}")
                nc.vector.tensor_copy(un[:, 0:512], av[0][:, :])
                nc.vector.tensor_copy(un[:, 512:1024], av[1][:, :])
                with nc.allow_low_precision("f32r rounding for PE streaming"):
                    nc.vector.reciprocal(rc[:, :], un[:, :])
                norm_state[ui] = (un, rc)

            def normalize_b(ui):
                ip, ihalf = units[ui]
                icol = ip * 1024 + ihalf * 512
                un, rc = norm_state.pop(ui)
                for h in range(2):
                    oT = oT0 if h == 0 else oT1
                    bc = avp.tile([64, 512], f32, tag="av", name=f"bc{ui}_{h}")
                    nc.tensor.matmul(
                        bc[:, :],
                        lhsT=ones65[64:65, :],
                        rhs=rc[64:65, h * 512:(h + 1) * 512],
                        start=True, stop=True,
                    )
                    nc.vector.tensor_mul(
                        oT[:, icol:icol + 512],
                        un[0:64, h * 512:(h + 1) * 512], bc[:, :])

            def out_proj_block(ii):
                po = avp.tile([128, 512], f32, tag="av", name=f"po{ii}")
                nc.tensor.matmul(po[:, :], lhsT=oT0[:, ii * 128:(ii + 1) * 128],
                                 rhs=wo0_sb[:, :], start=True, stop=False,
                                 skip_group_check=True)
                nc.tensor.matmul(po[:, :], lhsT=oT1[:, ii * 128:(ii + 1) * 128],
                                 rhs=wo1_sb[:, :], start=False, stop=True,
                                 skip_group_check=True)
                ob = obpool.tile([128, 512], f16, tag="ob", name=f"ob{ii}")
                nc.vector.tensor_copy(ob[:, :], po[:, :])
                nc.sync.dma_start(out[ii * 128:(ii + 1) * 128, :], ob[:, :])

            # ---- position-scheduled pipeline ----
            # priority within a position: 0=scores/exp, 1=extras, 2=AV
            import collections as _c
            sched = _c.defaultdict(list)

            def at(pos, prio, fn):
                sched[pos].append((prio, fn))

            AV_OFF = [12, 4, 4, 2]  # unit 0 trails the v transposes
            for ui in range(4):
                for jb in range(16):
                    at(ui * 16 + jb, 0, (lambda u, j: lambda: emit_scores_exp(u, j))(ui, jb))
                    at(ui * 16 + jb + AV_OFF[ui], 2, (lambda u, j: lambda: emit_av(u, j))(ui, jb))
                last_av = ui * 16 + 15 + AV_OFF[ui]
                at(last_av + 1, 1, (lambda u: lambda: normalize_a(u))(ui))
                at(last_av + 3, 1, (lambda u: lambda: normalize_b(u))(ui))

            # pre-unit-0 projections (emitted directly, not scheduled)
            # kT cols 0:1024 (jb 0-7) + qT cols 0:1024 come first; the rest weave in
            at(0, 1, lambda: proj_qk(kT, wk_sb, m01, 1, 0))
            at(1, 1, lambda: proj_qk(qT, wq_sb, x01, 1, 0))
            at(2, 1, lambda: proj_qk(vT, wv_sb, y0, 0, 0))
            at(3, 1, lambda: proj_qk(vT, wv_sb, y0, 1, 512))
            at(4, 1, lambda: proj_qk(kT, wk_sb, m1, 2, 0))
            at(5, 1, lambda: proj_qk(kT, wk_sb, m1, 3, 512))
            at(6, 1, lambda: proj_qk(vT, wv_sb, y1, 2, 0))
            at(7, 1, lambda: proj_qk(vT, wv_sb, y1, 3, 512))
            at(8, 1, lambda: [v_transpose(j) for j in range(4)])
            at(9, 1, lambda: [v_transpose(j) for j in range(4, 8)])
            at(10, 1, lambda: [v_transpose(j) for j in range(8, 12)])
            at(11, 1, lambda: [v_transpose(j) for j in range(12, 16)])
            at(18, 1, lambda: proj_qk(qT, wq_sb, x1, 2, 0))
            at(19, 1, lambda: proj_qk(qT, wq_sb, x1, 3, 512))
            # out-proj: block ii covers i-cols [ii*128, (ii+1)*128) which lie
            # inside unit ii//4's 512-wide i-block -> schedule right after that
            # unit's normalize_b
            for ii in range(16):
                uo = ii // 4
                pos = uo * 16 + 15 + AV_OFF[uo] + 5 + (ii % 4)
                at(pos, 1, (lambda b: lambda: out_proj_block(b))(ii))

            # projections gating unit 0
            proj_qk(kT, wk_sb, m00, 0, 0)
            proj_qk(qT, wq_sb, x00, 0, 0)

            for pos in sorted(sched):
                for _, fn in sorted(sched[pos], key=lambda t: t[0]):
                    fn()

    nc.compile()
    return nc


def _get_nc():
    if "nc" not in _CACHE:
        _CACHE["nc"] = _build_nc()
    return _CACHE["nc"]


def make_in_maps(x, m, y, Wq, Wk, Wv, Wo):
    """Shard full inputs into the 8 per-core input dicts."""
    x = np.asarray(x, dtype=np.float32)
    m = np.asarray(m, dtype=np.float32)
    y = np.asarray(y, dtype=np.float32)
    Wq = np.asarray(Wq, dtype=np.float32)
    Wk = np.asarray(Wk, dtype=np.float32)
    Wv = np.asarray(Wv, dtype=np.float32)
    Wo = np.asarray(Wo, dtype=np.float32)

    scale = np.float32(DIM ** -0.5)
    xT = np.ascontiguousarray(np.swapaxes(x, 1, 2))  # (B, DIM, N)
    mT = np.ascontiguousarray(np.swapaxes(m, 1, 2))
    yT = np.ascontiguousarray(np.swapaxes(y, 1, 2))

    xT = xT.astype(np.float16)
    mT = mT.astype(np.float16)
    yT = yT.astype(np.float16)
    in_maps = []
    for c in range(NCORES):
        bb, s = divmod(c, 4)
        sl = slice(s * SLICE, (s + 1) * SLICE)
        in_maps.append({
            "xT": xT[bb],
            "mT": mT[bb],
            "yT": yT[bb],
            "wq": np.ascontiguousarray(Wq[:, sl] * scale).astype(np.float16),
            "wk": np.ascontiguousarray(Wk[:, sl]).astype(np.float16),
            "wv": np.ascontiguousarray(Wv[:, sl]).astype(np.float16),
            "wo": np.ascontiguousarray(Wo[sl, :]),
        })
    return in_maps


def assemble(parts, bo):
    """Sum the 4 partial projections per batch and add bias."""
    bo = np.asarray(bo, dtype=np.float32)
    out = np.empty((B, N, DIM), dtype=np.float32)
    for bb in range(B):
        acc = np.zeros((N, DIM), dtype=np.float64)
        for s in range(4):
            acc += parts[bb * 4 + s]
        out[bb] = (acc + bo[None, :]).astype(np.float32)
    return out


def kernel(x, m, y, Wq, Wk, Wv, Wo, bo):
    from concourse.bass_utils import run_bass_kernel_spmd

    nc = _get_nc()
    in_maps = make_in_maps(x, m, y, Wq, Wk, Wv, Wo)
    trace = bool(os.environ.get("BASS_TRACE"))
    res = run_bass_kernel_spmd(nc, in_maps, list(range(NCORES)), trace=trace)
    _CACHE["last_results"] = res
    parts = [res.results[c]["out"] for c in range(NCORES)]
    return assemble(parts, bo)


# revision 42
# speedup vs baseline: 1.0261x; 1.0261x over previous
"""Trainium2 Bass kernel for cross-attention (b=2, n=2048, dim=512, 8 heads x 64).

Sharding: batch*heads across 8 cores. Core c handles batch c//4 and heads
{2*(c%4), 2*(c%4)+1} (a contiguous 128-col slice of the INNER dim).
Host pre-transposes activations to channel-major so the PE contraction dim
lands on partitions with fully-contiguous DMA.

Per-core dataflow (fp16 DMA/projection inputs, float32r PE streaming at
1 cycle/row, fp32 PSUM accumulation):
  qT/kT/vT = (W^T x) as [128 dcols, 2048 rows]; v re-transposed to [j, d]
  blocks via PE transpose, augmented with a ones column (denominator trick).
  Attention works on transposed scores so softmax needs no cross-partition
  reduction: scoresT[j,i] -> exp on ScalarE -> AV accumulate with
  lhsT=[v_h | 1] giving [64 d + 1 denom row, i] in PSUM.
  Normalize = reciprocal + K=1 ones-matmul broadcast + one multiply.
  Out-proj: partial = oT^T @ Wo_slice; host sums 4 partials/batch + bias.

Emission is a position-scheduled software pipeline paced by ScalarE (the
bottleneck engine): each position carries one scores-pair + one exp op;
AV matmuls trail by AV_OFF positions so the FIFO PE stream never blocks on
ScalarE or on late DMA arrivals; projections, v transposes, normalize and
out-proj blocks are slotted where their DMA/data dependencies land.
"""

import os
import numpy as np

DIM = 512
HEADS = 8
DH = 64
B = 2
N = 2048
NCORES = 8
SLICE = 128  # INNER columns per core (2 heads)

_CACHE = {}


def _build_nc():
    import concourse.bass as bass
    import concourse.mybir as mybir
    import concourse.tile as tile
    from concourse import bacc
    from concourse.masks import make_identity

    f32 = mybir.dt.float32
    f32r = mybir.dt.float32r
    f16 = mybir.dt.float16
    EXP = mybir.ActivationFunctionType.Exp

    nc = bacc.Bacc("TRN2", target_bir_lowering=False, debug=False)

    xT = nc.dram_tensor("xT", [DIM, N], f16, kind="ExternalInput").ap()
    mT = nc.dram_tensor("mT", [DIM, N], f16, kind="ExternalInput").ap()
    yT = nc.dram_tensor("yT", [DIM, N], f16, kind="ExternalInput").ap()
    wq = nc.dram_tensor("wq", [DIM, SLICE], f16, kind="ExternalInput").ap()
    wk = nc.dram_tensor("wk", [DIM, SLICE], f16, kind="ExternalInput").ap()
    wv = nc.dram_tensor("wv", [DIM, SLICE], f16, kind="ExternalInput").ap()
    wo = nc.dram_tensor("wo", [SLICE, DIM], f32r, kind="ExternalInput").ap()
    out = nc.dram_tensor("out", [N, DIM], f16, kind="ExternalOutput").ap()

    with tile.TileContext(nc) as tc:
        with (
            tc.tile_pool(name="wpool", bufs=1) as wpool,
            tc.tile_pool(name="cin1", bufs=4) as cin1,
            tc.tile_pool(name="persist", bufs=1) as persist,
            tc.tile_pool(name="expool", bufs=15) as expool,
            tc.tile_pool(name="snpool", bufs=2) as snpool,
            tc.tile_pool(name="obpool", bufs=3) as obpool,
            tc.tile_pool(name="mmp", bufs=2, space="PSUM") as mmp,
            tc.tile_pool(name="avp", bufs=4, space="PSUM") as avp,
        ):
            # ---- persistent SBUF tensors ----
            wq_sb = wpool.tile([128, DIM], f16, tag="wq")
            wk_sb = wpool.tile([128, DIM], f16, tag="wk")
            wv_sb = wpool.tile([128, DIM], f16, tag="wv")
            wo0_sb = wpool.tile([64, DIM], f32r, tag="wo0")
            wo1_sb = wpool.tile([64, DIM], f32r, tag="wo1")
            qT = persist.tile([128, N], f32r, tag="qT")
            kT = persist.tile([128, N], f32r, tag="kT")
            vT = persist.tile([128, N], f32r, tag="vT")
            ident_f = persist.tile([128, 128], f32, tag="ident_f")
            ident = persist.tile([128, 128], f32r, tag="ident")
            v_all = persist.tile([128, 16 * 130], f32r, tag="v_all")
            oT0 = persist.tile([64, N], f32r, tag="oT0")
            oT1 = persist.tile([64, N], f32r, tag="oT1")
            ones65 = persist.tile([65, 64], f32r, tag="ones65")
            ones_f = persist.tile([128, 64], f32, tag="ones_f")
            zbias = persist.tile([128, 1], f32, tag="zbias")

            nc.vector.memset(ones_f[:, :], 1.0)
            nc.vector.memset(zbias[:, :], 0.0)
            nc.vector.tensor_copy(ones65[:, :], ones_f[0:65, :])
            nc.vector.tensor_copy(v_all[:, 64::65], ones_f[:, 0:32])
            make_identity(nc, ident_f[:, :])
            nc.vector.tensor_copy(ident[:, :], ident_f[:, :])

            def load_w(w_dram, w_sb):
                nc.sync.dma_start(
                    w_sb[:, :].rearrange("p (c m) -> p c m", c=4),
                    w_dram.rearrange("(c p) m -> p c m", p=128),
                )

            # ---- input DMAs: column-halves, queue order = priority ----
            # order: m[col0], x[col0], y[col0], y[col1], m[col1], x[col1]
            # one DMA per 4-chunk column group: dst packs the 4 channel-chunks
            # side by side in the free dim; src gathers with a 3D AP (the same
            # pattern as load_w). Fewer, larger DMAs = less queue overhead on
            # the critical startup path.
            def load_group(src_ap, col0, width, name, tag):
                t = cin1.tile([128, 4 * width], f16, tag=tag, name=name)
                nc.sync.dma_start(
                    t[:, :].rearrange("p (c f) -> p c f", c=4),
                    src_ap[:, col0:col0 + width].rearrange("(c p) f -> p c f", p=128),
                )
                return [t[:, ck * width:(ck + 1) * width] for ck in range(4)]

            def load_half(src_ap, col, name):
                return load_group(src_ap, col * 1024, 1024, f"{name}h{col}", "cin")

            def load_quarter(src_ap, q, name):
                return load_group(src_ap, q * 512, 512, f"{name}q{q}", "cinq")

            load_w(wk, wk_sb)
            m00 = load_quarter(mT, 0, "m")
            load_w(wq, wq_sb)
            x00 = load_quarter(xT, 0, "x")
            m01 = load_quarter(mT, 1, "m")
            x01 = load_quarter(xT, 1, "x")
            load_w(wv, wv_sb)
            y0 = load_half(yT, 0, "y")
            m1 = load_half(mT, 1, "m")
            y1 = load_half(yT, 1, "y")
            x1 = load_half(xT, 1, "x")
            nc.sync.dma_start(wo0_sb[:, :], wo[0:64, :])
            nc.sync.dma_start(wo1_sb[:, :], wo[64:128, :])

            # ---- building blocks ----
            def proj_qk(dst, w_sb, chunks, nb, chunk_col_off):
                ps = avp.tile([128, 512], f32, tag="av", name=f"ps_p{dst.tensor.name}_{nb}")
                for ck in range(4):
                    nc.tensor.matmul(
                        ps[:, :],
                        lhsT=w_sb[:, ck * 128:(ck + 1) * 128],
                        rhs=chunks[ck][:, chunk_col_off:chunk_col_off + 512],
                        start=(ck == 0), stop=(ck == 3),
                    )
                nc.vector.tensor_copy(dst[:, nb * 512:(nb + 1) * 512], ps[:, :])

            def v_transpose(jb):
                pv = avp.tile([128, 512], f32r, tag="av", name=f"pv{jb}")
                nc.tensor.transpose(pv[:, 0:128], vT[:, jb * 128:(jb + 1) * 128], ident[:, :])
                base = jb * 130
                nc.vector.tensor_copy(v_all[:, base:base + 64], pv[:, 0:64])
                nc.vector.tensor_copy(v_all[:, base + 65:base + 129], pv[:, 64:128])

            # per-unit state: unit = (ip, ihalf); the 1024-wide psum/exp tile
            # packs [head0 | head1] for one 512-wide i-block, so the two score
            # matmuls sit on disjoint PE row groups (base partitions 0/64) and
            # overlap in the array.
            units = [(0, 0), (0, 1), (1, 0), (1, 1)]
            av_tiles = {}   # ui -> [av0, av1]
            ex_tiles = {}   # (ui, jb) -> exp tile
            norm_state = {}

            def emit_scores_exp(ui, jb):
                ip, ihalf = units[ui]
                icol = ip * 1024 + ihalf * 512
                ps = mmp.tile([128, 1024], f32, tag="mm", name=f"ps_s{ui}_{jb}")
                for h in range(2):
                    hp = h * 64
                    nc.tensor.matmul(
                        ps[:, h * 512:(h + 1) * 512],
                        lhsT=kT[hp:hp + 64, jb * 128:(jb + 1) * 128],
                        rhs=qT[hp:hp + 64, icol:icol + 512],
                        start=True, stop=True,
                    )
                ex = expool.tile([128, 1024], f32r, tag="ex", name=f"ex{ui}_{jb}")
                nc.scalar.activation(ex[:, :], ps[:, :], EXP, bias=zbias[:, 0:1])
                ex_tiles[(ui, jb)] = ex

            def emit_av(ui, jb):
                if jb == 0:
                    av_tiles[ui] = [
                        avp.tile([65, 512], f32, tag="av", name=f"av{ui}_{h}")
                        for h in range(2)
                    ]
                av = av_tiles[ui]
                ex = ex_tiles.pop((ui, jb))
                for h in range(2):
                    vb = jb * 130 + h * 65
                    nc.tensor.matmul(
                        av[h][:, :],
                        lhsT=v_all[:, vb:vb + 65],
                        rhs=ex[:, h * 512:(h + 1) * 512],
                        start=(jb == 0), stop=(jb == 15),
                        skip_group_check=True,
                    )

            def normalize_a(ui):
                av = av_tiles[ui]
                un = snpool.tile([65, 1024], f32, tag="un", name=f"un{ui}")
                rc = snpool.tile([65, 1024], f32r, tag="rc", name=f"rc{ui}")
                # reciprocals straight from PSUM so normalize_b's broadcast
                # matmul doesn't wait for the un copies
                with nc.allow_low_precision("f32r rounding for PE streaming"):
                    nc.vector.reciprocal(rc[:, 0:512], av[0][:, :])
                    nc.vector.reciprocal(rc[:, 512:1024], av[1][:, :])
                nc.vector.tensor_copy(un[:, 0:512], av[0][:, :])
                nc.vector.tensor_copy(un[:, 512:1024], av[1][:, :])
                norm_state[ui] = (un, rc)

            def normalize_b(ui):
                ip, ihalf = units[ui]
                icol = ip * 1024 + ihalf * 512
                un, rc = norm_state.pop(ui)
                for h in range(2):
                    oT = oT0 if h == 0 else oT1
                    bc = avp.tile([64, 512], f32, tag="av", name=f"bc{ui}_{h}")
                    nc.tensor.matmul(
                        bc[:, :],
                        lhsT=ones65[64:65, :],
                        rhs=rc[64:65, h * 512:(h + 1) * 512],
                        start=True, stop=True,
                    )
                    nc.vector.tensor_mul(
                        oT[:, icol:icol + 512],
                        un[0:64, h * 512:(h + 1) * 512], bc[:, :])

            def out_proj_block(ii):
                po = avp.tile([128, 512], f32, tag="av", name=f"po{ii}")
                nc.tensor.matmul(po[:, :], lhsT=oT0[:, ii * 128:(ii + 1) * 128],
                                 rhs=wo0_sb[:, :], start=True, stop=False,
                                 skip_group_check=True)
                nc.tensor.matmul(po[:, :], lhsT=oT1[:, ii * 128:(ii + 1) * 128],
                                 rhs=wo1_sb[:, :], start=False, stop=True,
                                 skip_group_check=True)
                ob = obpool.tile([128, 512], f16, tag="ob", name=f"ob{ii}")
                nc.vector.tensor_copy(ob[:, :], po[:, :])
                nc.sync.dma_start(out[ii * 128:(ii + 1) * 128, :], ob[:, :])

            # ---- position-scheduled pipeline ----
            # priority within a position: 0=scores/exp, 1=extras, 2=AV
            import collections as _c
            sched = _c.defaultdict(list)

            def at(pos, prio, fn):
                sched[pos].append((prio, fn))

            AV_OFF = [12, 4, 4, 2]  # unit 0 trails the v transposes
            for ui in range(4):
                for jb in range(16):
                    at(ui * 16 + jb, 0, (lambda u, j: lambda: emit_scores_exp(u, j))(ui, jb))
                    # last unit: let the final AVs trail by one position only --
                    # nothing else needs the PE once the exp stream ends
                    off = 1 if (ui == 3 and jb >= 14) else AV_OFF[ui]
                    at(ui * 16 + jb + off, 2, (lambda u, j: lambda: emit_av(u, j))(ui, jb))
                last_av = ui * 16 + 15 + (1 if ui == 3 else AV_OFF[ui])
                at(last_av + 1, 1, (lambda u: lambda: normalize_a(u))(ui))
                at(last_av + 3, 1, (lambda u: lambda: normalize_b(u))(ui))

            # pre-unit-0 projections (emitted directly, not scheduled)
            # kT cols 0:1024 (jb 0-7) + qT cols 0:1024 come first; the rest weave in
            at(0, 1, lambda: proj_qk(kT, wk_sb, m01, 1, 0))
            at(1, 1, lambda: proj_qk(qT, wq_sb, x01, 1, 0))
            at(2, 1, lambda: proj_qk(vT, wv_sb, y0, 0, 0))
            at(3, 1, lambda: proj_qk(vT, wv_sb, y0, 1, 512))
            at(4, 1, lambda: proj_qk(kT, wk_sb, m1, 2, 0))
            at(5, 1, lambda: proj_qk(kT, wk_sb, m1, 3, 512))
            at(6, 1, lambda: proj_qk(vT, wv_sb, y1, 2, 0))
            at(7, 1, lambda: proj_qk(vT, wv_sb, y1, 3, 512))
            at(8, 1, lambda: [v_transpose(j) for j in range(4)])
            at(9, 1, lambda: [v_transpose(j) for j in range(4, 8)])
            at(10, 1, lambda: [v_transpose(j) for j in range(8, 12)])
            at(11, 1, lambda: [v_transpose(j) for j in range(12, 16)])
            at(18, 1, lambda: proj_qk(qT, wq_sb, x1, 2, 0))
            at(19, 1, lambda: proj_qk(qT, wq_sb, x1, 3, 512))
            # out-proj: block ii covers i-cols [ii*128, (ii+1)*128) which lie
            # inside unit ii//4's 512-wide i-block -> schedule right after that
            # unit's normalize_b
            for ii in range(16):
                uo = ii // 4
                pos = uo * 16 + 15 + AV_OFF[uo] + 5 + (ii % 4)
                at(pos, 1, (lambda b: lambda: out_proj_block(b))(ii))

            # projections gating unit 0
            proj_qk(kT, wk_sb, m00, 0, 0)
            proj_qk(qT, wq_sb, x00, 0, 0)

            for pos in sorted(sched):
                for _, fn in sorted(sched[pos], key=lambda t: t[0]):
                    fn()

    nc.compile()
    return nc


def _get_nc():
    if "nc" not in _CACHE:
        _CACHE["nc"] = _build_nc()
    return _CACHE["nc"]


def make_in_maps(x, m, y, Wq, Wk, Wv, Wo):
    """Shard full inputs into the 8 per-core input dicts."""
    x = np.asarray(x, dtype=np.float32)
    m = np.asarray(m, dtype=np.float32)
    y = np.asarray(y, dtype=np.float32)
    Wq = np.asarray(Wq, dtype=np.float32)
    Wk = np.asarray(Wk, dtype=np.float32)
    Wv = np.asarray(Wv, dtype=np.float32)
    Wo = np.asarray(Wo, dtype=np.float32)

    scale = np.float32(DIM ** -0.5)
    xT = np.ascontiguousarray(np.swapaxes(x, 1, 2))  # (B, DIM, N)
    mT = np.ascontiguousarray(np.swapaxes(m, 1, 2))
    yT = np.ascontiguousarray(np.swapaxes(y, 1, 2))

    xT = xT.astype(np.float16)
    mT = mT.astype(np.float16)
    yT = yT.astype(np.float16)
    in_maps = []
    for c in range(NCORES):
        bb, s = divmod(c, 4)
        sl = slice(s * SLICE, (s + 1) * SLICE)
        in_maps.append({
            "xT": xT[bb],
            "mT": mT[bb],
            "yT": yT[bb],
            "wq": np.ascontiguousarray(Wq[:, sl] * scale).astype(np.float16),
            "wk": np.ascontiguousarray(Wk[:, sl]).astype(np.float16),
            "wv": np.ascontiguousarray(Wv[:, sl]).astype(np.float16),
            "wo": np.ascontiguousarray(Wo[sl, :]),
        })
    return in_maps


def assemble(parts, bo):
    """Sum the 4 partial projections per batch and add bias."""
    bo = np.asarray(bo, dtype=np.float32)
    out = np.empty((B, N, DIM), dtype=np.float32)
    for bb in range(B):
        acc = np.zeros((N, DIM), dtype=np.float64)
        for s in range(4):
            acc += parts[bb * 4 + s]
        out[bb] = (acc + bo[None, :]).astype(np.float32)
    return out


def kernel(x, m, y, Wq, Wk, Wv, Wo, bo):
    from concourse.bass_utils import run_bass_kernel_spmd

    nc = _get_nc()
    in_maps = make_in_maps(x, m, y, Wq, Wk, Wv, Wo)
    trace = bool(os.environ.get("BASS_TRACE"))
    res = run_bass_kernel_spmd(nc, in_maps, list(range(NCORES)), trace=trace)
    _CACHE["last_results"] = res
    parts = [res.results[c]["out"] for c in range(NCORES)]
    return assemble(parts, bo)


# revision 43
# speedup vs baseline: 1.0303x; 1.0040x over previous
"""Trainium2 Bass kernel for cross-attention (b=2, n=2048, dim=512, 8 heads x 64).

Sharding: batch*heads across 8 cores. Core c handles batch c//4 and heads
{2*(c%4), 2*(c%4)+1} (a contiguous 128-col slice of the INNER dim).
Host pre-transposes activations to channel-major so the PE contraction dim
lands on partitions with fully-contiguous DMA.

Per-core dataflow (fp16 DMA/projection inputs, float32r PE streaming at
1 cycle/row, fp32 PSUM accumulation):
  qT/kT/vT = (W^T x) as [128 dcols, 2048 rows]; v re-transposed to [j, d]
  blocks via PE transpose, augmented with a ones column (denominator trick).
  Attention works on transposed scores so softmax needs no cross-partition
  reduction: scoresT[j,i] -> exp on ScalarE -> AV accumulate with
  lhsT=[v_h | 1] giving [64 d + 1 denom row, i] in PSUM.
  Normalize = reciprocal + K=1 ones-matmul broadcast + one multiply.
  Out-proj: partial = oT^T @ Wo_slice; host sums 4 partials/batch + bias.

Emission is a position-scheduled software pipeline paced by ScalarE (the
bottleneck engine): each position carries one scores-pair + one exp op;
AV matmuls trail by AV_OFF positions so the FIFO PE stream never blocks on
ScalarE or on late DMA arrivals; projections, v transposes, normalize and
out-proj blocks are slotted where their DMA/data dependencies land.
"""

import os
import numpy as np

DIM = 512
HEADS = 8
DH = 64
B = 2
N = 2048
NCORES = 8
SLICE = 128  # INNER columns per core (2 heads)

_CACHE = {}


def _build_nc():
    import concourse.bass as bass
    import concourse.mybir as mybir
    import concourse.tile as tile
    from concourse import bacc
    from concourse.masks import make_identity

    f32 = mybir.dt.float32
    f32r = mybir.dt.float32r
    f16 = mybir.dt.float16
    EXP = mybir.ActivationFunctionType.Exp

    nc = bacc.Bacc("TRN2", target_bir_lowering=False, debug=False)

    xT = nc.dram_tensor("xT", [DIM, N], f16, kind="ExternalInput").ap()
    mT = nc.dram_tensor("mT", [DIM, N], f16, kind="ExternalInput").ap()
    yT = nc.dram_tensor("yT", [DIM, N], f16, kind="ExternalInput").ap()
    wq = nc.dram_tensor("wq", [DIM, SLICE], f16, kind="ExternalInput").ap()
    wk = nc.dram_tensor("wk", [DIM, SLICE], f16, kind="ExternalInput").ap()
    wv = nc.dram_tensor("wv", [DIM, SLICE], f16, kind="ExternalInput").ap()
    wo = nc.dram_tensor("wo", [SLICE, DIM], f32r, kind="ExternalInput").ap()
    out = nc.dram_tensor("out", [N, DIM], f16, kind="ExternalOutput").ap()

    with tile.TileContext(nc) as tc:
        with (
            tc.tile_pool(name="wpool", bufs=1) as wpool,
            tc.tile_pool(name="cin1", bufs=4) as cin1,
            tc.tile_pool(name="persist", bufs=1) as persist,
            tc.tile_pool(name="expool", bufs=15) as expool,
            tc.tile_pool(name="snpool", bufs=2) as snpool,
            tc.tile_pool(name="obpool", bufs=2) as obpool,
            tc.tile_pool(name="mmp", bufs=2, space="PSUM") as mmp,
            tc.tile_pool(name="avp", bufs=4, space="PSUM") as avp,
        ):
            # ---- persistent SBUF tensors ----
            wq_sb = wpool.tile([128, DIM], f16, tag="wq")
            wk_sb = wpool.tile([128, DIM], f16, tag="wk")
            wv_sb = wpool.tile([128, DIM], f16, tag="wv")
            wo0_sb = wpool.tile([64, DIM], f32r, tag="wo0")
            wo1_sb = wpool.tile([64, DIM], f32r, tag="wo1")
            qT = persist.tile([128, N], f32r, tag="qT")
            kT = persist.tile([128, N], f32r, tag="kT")
            vT = persist.tile([128, N], f32r, tag="vT")
            ident_f = persist.tile([128, 128], f32, tag="ident_f")
            ident = persist.tile([128, 128], f32r, tag="ident")
            v_all = persist.tile([128, 16 * 130], f32r, tag="v_all")
            oT0 = persist.tile([64, N], f32r, tag="oT0")
            oT1 = persist.tile([64, N], f32r, tag="oT1")
            ones65 = persist.tile([65, 64], f32r, tag="ones65")
            ones_f = persist.tile([128, 64], f32, tag="ones_f")
            zbias = persist.tile([128, 1], f32, tag="zbias")

            nc.vector.memset(ones_f[:, :], 1.0)
            nc.vector.memset(zbias[:, :], 0.0)
            nc.vector.tensor_copy(ones65[:, :], ones_f[0:65, :])
            nc.vector.tensor_copy(v_all[:, 64::65], ones_f[:, 0:32])
            make_identity(nc, ident_f[:, :])
            nc.vector.tensor_copy(ident[:, :], ident_f[:, :])

            def load_w(w_dram, w_sb):
                nc.sync.dma_start(
                    w_sb[:, :].rearrange("p (c m) -> p c m", c=4),
                    w_dram.rearrange("(c p) m -> p c m", p=128),
                )

            # ---- input DMAs: column-halves, queue order = priority ----
            # order: m[col0], x[col0], y[col0], y[col1], m[col1], x[col1]
            # one DMA per 4-chunk column group: dst packs the 4 channel-chunks
            # side by side in the free dim; src gathers with a 3D AP (the same
            # pattern as load_w). Fewer, larger DMAs = less queue overhead on
            # the critical startup path.
            def load_group(src_ap, col0, width, name, tag):
                t = cin1.tile([128, 4 * width], f16, tag=tag, name=name)
                nc.sync.dma_start(
                    t[:, :].rearrange("p (c f) -> p c f", c=4),
                    src_ap[:, col0:col0 + width].rearrange("(c p) f -> p c f", p=128),
                )
                return [t[:, ck * width:(ck + 1) * width] for ck in range(4)]

            def load_half(src_ap, col, name):
                return load_group(src_ap, col * 1024, 1024, f"{name}h{col}", "cin")

            def load_quarter(src_ap, q, name):
                return load_group(src_ap, q * 512, 512, f"{name}q{q}", "cinq")

            load_w(wk, wk_sb)
            m00 = load_quarter(mT, 0, "m")
            load_w(wq, wq_sb)
            x00 = load_quarter(xT, 0, "x")
            m01 = load_quarter(mT, 1, "m")
            x01 = load_quarter(xT, 1, "x")
            load_w(wv, wv_sb)
            y0 = load_half(yT, 0, "y")
            m1 = load_half(mT, 1, "m")
            y1 = load_half(yT, 1, "y")
            x1 = load_half(xT, 1, "x")
            nc.sync.dma_start(wo0_sb[:, :], wo[0:64, :])
            nc.sync.dma_start(wo1_sb[:, :], wo[64:128, :])

            # ---- building blocks ----
            def proj_qk(dst, w_sb, chunks, nb, chunk_col_off):
                ps = avp.tile([128, 512], f32, tag="av", name=f"ps_p{dst.tensor.name}_{nb}")
                for ck in range(4):
                    nc.tensor.matmul(
                        ps[:, :],
                        lhsT=w_sb[:, ck * 128:(ck + 1) * 128],
                        rhs=chunks[ck][:, chunk_col_off:chunk_col_off + 512],
                        start=(ck == 0), stop=(ck == 3),
                    )
                nc.vector.tensor_copy(dst[:, nb * 512:(nb + 1) * 512], ps[:, :])

            def v_transpose(jb):
                pv = avp.tile([128, 512], f32r, tag="av", name=f"pv{jb}")
                nc.tensor.transpose(pv[:, 0:128], vT[:, jb * 128:(jb + 1) * 128], ident[:, :])
                base = jb * 130
                nc.vector.tensor_copy(v_all[:, base:base + 64], pv[:, 0:64])
                nc.vector.tensor_copy(v_all[:, base + 65:base + 129], pv[:, 64:128])

            # per-unit state: unit = (ip, ihalf); the 1024-wide psum/exp tile
            # packs [head0 | head1] for one 512-wide i-block, so the two score
            # matmuls sit on disjoint PE row groups (base partitions 0/64) and
            # overlap in the array.
            units = [(0, 0), (0, 1), (1, 0), (1, 1)]
            av_tiles = {}   # ui -> [av0, av1]
            ex_tiles = {}   # (ui, jb) -> exp tile
            norm_state = {}

            def emit_scores_exp(ui, jb):
                ip, ihalf = units[ui]
                icol = ip * 1024 + ihalf * 512
                ps = mmp.tile([128, 1024], f32, tag="mm", name=f"ps_s{ui}_{jb}")
                for h in range(2):
                    hp = h * 64
                    nc.tensor.matmul(
                        ps[:, h * 512:(h + 1) * 512],
                        lhsT=kT[hp:hp + 64, jb * 128:(jb + 1) * 128],
                        rhs=qT[hp:hp + 64, icol:icol + 512],
                        start=True, stop=True,
                    )
                ex = expool.tile([128, 1024], f32r, tag="ex", name=f"ex{ui}_{jb}")
                nc.scalar.activation(ex[:, :], ps[:, :], EXP, bias=zbias[:, 0:1])
                ex_tiles[(ui, jb)] = ex

            def emit_av(ui, jb):
                if jb == 0:
                    av_tiles[ui] = [
                        avp.tile([65, 512], f32, tag="av", name=f"av{ui}_{h}")
                        for h in range(2)
                    ]
                av = av_tiles[ui]
                ex = ex_tiles.pop((ui, jb))
                for h in range(2):
                    vb = jb * 130 + h * 65
                    nc.tensor.matmul(
                        av[h][:, :],
                        lhsT=v_all[:, vb:vb + 65],
                        rhs=ex[:, h * 512:(h + 1) * 512],
                        start=(jb == 0), stop=(jb == 15),
                        skip_group_check=True,
                    )

            def normalize_a(ui):
                av = av_tiles[ui]
                un = snpool.tile([65, 1024], f32, tag="un", name=f"un{ui}")
                rc = snpool.tile([65, 1024], f32r, tag="rc", name=f"rc{ui}")
                # reciprocals straight from PSUM so normalize_b's broadcast
                # matmul doesn't wait for the un copies
                with nc.allow_low_precision("f32r rounding for PE streaming"):
                    nc.vector.reciprocal(rc[:, 0:512], av[0][:, :])
                    nc.vector.reciprocal(rc[:, 512:1024], av[1][:, :])
                nc.vector.tensor_copy(un[:, 0:512], av[0][:, :])
                nc.vector.tensor_copy(un[:, 512:1024], av[1][:, :])
                norm_state[ui] = (un, rc)

            def normalize_b(ui):
                ip, ihalf = units[ui]
                icol = ip * 1024 + ihalf * 512
                un, rc = norm_state.pop(ui)
                for h in range(2):
                    oT = oT0 if h == 0 else oT1
                    bc = avp.tile([64, 512], f32, tag="av", name=f"bc{ui}_{h}")
                    nc.tensor.matmul(
                        bc[:, :],
                        lhsT=ones65[64:65, :],
                        rhs=rc[64:65, h * 512:(h + 1) * 512],
                        start=True, stop=True,
                    )
                    nc.vector.tensor_mul(
                        oT[:, icol:icol + 512],
                        un[0:64, h * 512:(h + 1) * 512], bc[:, :])

            ob_tiles = {}

            def out_proj_block(ii):
                uo, b = divmod(ii, 4)
                if b == 0:
                    ob_tiles[uo] = obpool.tile([128, 2048], f16, tag="ob",
                                               name=f"ob{uo}")
                ob = ob_tiles[uo]
                po = avp.tile([128, 512], f32, tag="av", name=f"po{ii}")
                nc.tensor.matmul(po[:, :], lhsT=oT0[:, ii * 128:(ii + 1) * 128],
                                 rhs=wo0_sb[:, :], start=True, stop=False,
                                 skip_group_check=True)
                nc.tensor.matmul(po[:, :], lhsT=oT1[:, ii * 128:(ii + 1) * 128],
                                 rhs=wo1_sb[:, :], start=False, stop=True,
                                 skip_group_check=True)
                nc.vector.tensor_copy(ob[:, b * 512:(b + 1) * 512], po[:, :])
                if b == 3:
                    # one wide store per 512-row group (4 blocks packed in ob)
                    nc.sync.dma_start(
                        out[uo * 512:(uo + 1) * 512, :].rearrange("(b p) e -> p b e", p=128),
                        ob_tiles.pop(uo)[:, :].rearrange("p (b e) -> p b e", b=4),
                    )

            # ---- position-scheduled pipeline ----
            # priority within a position: 0=scores/exp, 1=extras, 2=AV
            import collections as _c
            sched = _c.defaultdict(list)

            def at(pos, prio, fn):
                sched[pos].append((prio, fn))

            AV_OFF = [12, 4, 4, 2]  # unit 0 trails the v transposes
            for ui in range(4):
                for jb in range(16):
                    at(ui * 16 + jb, 0, (lambda u, j: lambda: emit_scores_exp(u, j))(ui, jb))
                    # last unit: let the final AVs trail by one position only --
                    # nothing else needs the PE once the exp stream ends
                    off = 1 if (ui == 3 and jb >= 14) else AV_OFF[ui]
                    at(ui * 16 + jb + off, 2, (lambda u, j: lambda: emit_av(u, j))(ui, jb))
                last_av = ui * 16 + 15 + (1 if ui == 3 else AV_OFF[ui])
                at(last_av + 1, 1, (lambda u: lambda: normalize_a(u))(ui))
                at(last_av + 3, 1, (lambda u: lambda: normalize_b(u))(ui))

            # pre-unit-0 projections (emitted directly, not scheduled)
            # kT cols 0:1024 (jb 0-7) + qT cols 0:1024 come first; the rest weave in
            at(0, 1, lambda: proj_qk(kT, wk_sb, m01, 1, 0))
            at(1, 1, lambda: proj_qk(qT, wq_sb, x01, 1, 0))
            at(2, 1, lambda: proj_qk(vT, wv_sb, y0, 0, 0))
            at(3, 1, lambda: proj_qk(vT, wv_sb, y0, 1, 512))
            at(4, 1, lambda: proj_qk(kT, wk_sb, m1, 2, 0))
            at(5, 1, lambda: proj_qk(kT, wk_sb, m1, 3, 512))
            at(6, 1, lambda: proj_qk(vT, wv_sb, y1, 2, 0))
            at(7, 1, lambda: proj_qk(vT, wv_sb, y1, 3, 512))
            at(8, 1, lambda: [v_transpose(j) for j in range(4)])
            at(9, 1, lambda: [v_transpose(j) for j in range(4, 8)])
            at(10, 1, lambda: [v_transpose(j) for j in range(8, 12)])
            at(11, 1, lambda: [v_transpose(j) for j in range(12, 16)])
            at(18, 1, lambda: proj_qk(qT, wq_sb, x1, 2, 0))
            at(19, 1, lambda: proj_qk(qT, wq_sb, x1, 3, 512))
            # out-proj: block ii covers i-cols [ii*128, (ii+1)*128) which lie
            # inside unit ii//4's 512-wide i-block -> schedule right after that
            # unit's normalize_b
            for ii in range(16):
                uo = ii // 4
                pos = uo * 16 + 15 + AV_OFF[uo] + 5 + (ii % 4)
                at(pos, 1, (lambda b: lambda: out_proj_block(b))(ii))

            # projections gating unit 0
            proj_qk(kT, wk_sb, m00, 0, 0)
            proj_qk(qT, wq_sb, x00, 0, 0)

            for pos in sorted(sched):
                for _, fn in sorted(sched[pos], key=lambda t: t[0]):
                    fn()

    nc.compile()
    return nc


def _get_nc():
    if "nc" not in _CACHE:
        _CACHE["nc"] = _build_nc()
    return _CACHE["nc"]


def make_in_maps(x, m, y, Wq, Wk, Wv, Wo):
    """Shard full inputs into the 8 per-core input dicts."""
    x = np.asarray(x, dtype=np.float32)
    m = np.asarray(m, dtype=np.float32)
    y = np.asarray(y, dtype=np.float32)
    Wq = np.asarray(Wq, dtype=np.float32)
    Wk = np.asarray(Wk, dtype=np.float32)
    Wv = np.asarray(Wv, dtype=np.float32)
    Wo = np.asarray(Wo, dtype=np.float32)

    scale = np.float32(DIM ** -0.5)
    xT = np.ascontiguousarray(np.swapaxes(x, 1, 2))  # (B, DIM, N)
    mT = np.ascontiguousarray(np.swapaxes(m, 1, 2))
    yT = np.ascontiguousarray(np.swapaxes(y, 1, 2))

    xT = xT.astype(np.float16)
    mT = mT.astype(np.float16)
    yT = yT.astype(np.float16)
    in_maps = []
    for c in range(NCORES):
        bb, s = divmod(c, 4)
        sl = slice(s * SLICE, (s + 1) * SLICE)
        in_maps.append({
            "xT": xT[bb],
            "mT": mT[bb],
            "yT": yT[bb],
            "wq": np.ascontiguousarray(Wq[:, sl] * scale).astype(np.float16),
            "wk": np.ascontiguousarray(Wk[:, sl]).astype(np.float16),
            "wv": np.ascontiguousarray(Wv[:, sl]).astype(np.float16),
            "wo": np.ascontiguousarray(Wo[sl, :]),
        })
    return in_maps


def assemble(parts, bo):
    """Sum the 4 partial projections per batch and add bias."""
    bo = np.asarray(bo, dtype=np.float32)
    out = np.empty((B, N, DIM), dtype=np.float32)
    for bb in range(B):
        acc = np.zeros((N, DIM), dtype=np.float64)
        for s in range(4):
            acc += parts[bb * 4 + s]
        out[bb] = (acc + bo[None, :]).astype(np.float32)
    return out


def kernel(x, m, y, Wq, Wk, Wv, Wo, bo):
    from concourse.bass_utils import run_bass_kernel_spmd

    nc = _get_nc()
    in_maps = make_in_maps(x, m, y, Wq, Wk, Wv, Wo)
    trace = bool(os.environ.get("BASS_TRACE"))
    res = run_bass_kernel_spmd(nc, in_maps, list(range(NCORES)), trace=trace)
    _CACHE["last_results"] = res
    parts = [res.results[c]["out"] for c in range(NCORES)]
    return assemble(parts, bo)
